# revision 3
# baseline (speedup 1.0000x reference)
"""Trainium2 Bass kernel for a 5x5 valid convolution over 96x96 images.

Reference computes x @ W.T where W is the [8464, 9216] conv-as-matmul
matrix (10 GFLOP dense).  We compute the convolution directly on the
tensor engine as 5 PSUM-accumulated banded matmuls (row-conv over the
image-row contraction, column shifts folded into the rhs access pattern):

    out[oi, b, oj] = sum_kj  B_kj.T @ X[:, b, oj+kj]
    B_kj[i, oi]    = K[i-oi, kj]   (banded Toeplitz, built on device)

Sharding: data-parallel over batch; each of the 8 cores convolves 8
images.  Raw Bass without a Block, hand-scheduled static DAG.
Schedule: x load split across both HWDGE rings; the 25-tap scatter and
the banded Toeplitz load are single merged DMAs; the band reversal is
one strided cast; PSUM->SBUF quarter copies run on Pool so DVE only
does the casts; stores alternate rings and issue as quarters complete.
"""

import sys

sys.path.insert(0, "/opt/trn_rl_repo")

import numpy as np

import bass_rust
import concourse.bass as bass
import concourse.mybir as mybir
from concourse.bass_utils import run_bass_kernel_spmd

# Problem geometry (hardcoded per the task contract).
BATCH = 64
IN = 96           # input image side
KD = 5            # conv kernel side
OD = IN - KD + 1  # output side = 92
ISIZE = IN * IN   # 9216
OSIZE = OD * OD   # 8464
NCORES = 8
BPC = BATCH // NCORES  # images per core = 8
HALF = BPC // 2        # images per PSUM accumulation group = 4
QTR = BPC // 4         # images per store quarter = 2
UL = 187               # per-kj stripe length in the padded tap vector u


def _ap(view, offset, dims):
    ap = view.copy()
    ap.offset = offset
    ap.ap = bass_rust.VecI64Pair(dims)
    return ap


def _build_program():
    nc = bass.Bass()
    dt = mybir.dt.float32
    f32r = mybir.dt.float32r

    x_in = nc.declare_dram_parameter("x", [BPC, ISIZE], dt, isOutput=False)
    k_in = nc.declare_dram_parameter("k", [KD, KD], dt, isOutput=False)
    y_out = nc.declare_dram_parameter("y", [BPC, OSIZE], dt, isOutput=True)
    # Zero-initialized at NEFF load (static DMA runs during the engine
    # preamble, off the critical path); per-run the scatter overwrites
    # all 25 tap positions, so repeated executions stay correct.
    u_dram = nc.inline_tensor(np.zeros(KD * UL, np.float32), "u_scratch")

    from contextlib import ExitStack

    with ExitStack() as ctx:
        b_tmp = ctx.enter_context(nc.sbuf_tensor("b_tmp", [IN, KD, OD], dt))
        b_sb = ctx.enter_context(nc.sbuf_tensor("b_sb", [IN, KD, OD], f32r))
        x_sb = ctx.enter_context(nc.sbuf_tensor("x_sb", [IN, BPC, IN], dt))
        x_r = ctx.enter_context(nc.sbuf_tensor("x_r", [IN, BPC, IN], f32r))
        out_sb = ctx.enter_context(nc.sbuf_tensor("out_sb", [OD, BPC, OD], dt))
        ps0 = ctx.enter_context(nc.psum_tensor("ps0", [OD, HALF, OD], dt))
        ps1 = ctx.enter_context(nc.psum_tensor("ps1", [OD, HALF, OD], dt))
        sem = lambda n: ctx.enter_context(nc.semaphore(n))
        sem_scat = sem("sem_scat")    # 25 taps scattered into u_dram
        sem_x0 = sem("sem_x0")        # images 0..3 -> x_sb
        sem_x1 = sem("sem_x1")        # images 4..7 -> x_sb
        sem_bt = sem("sem_bt")        # banded load u -> b_tmp (all stripes)
        sem_xr0 = sem("sem_xr0")      # x half 0 rounded to f32r
        sem_xr1 = sem("sem_xr1")      # x half 1 rounded to f32r
        sem_brev = sem("sem_brev")    # band reversed+cast -> b_sb
        sem_mm = sem("sem_mm")        # psum group done
        sem_copy = sem("sem_copy")    # psum -> out_sb quarter done
        sem_y = sem("sem_y")          # out_sb -> y

        psums = [ps0, ps1]

        # ---- sync ring: tap scatter, x half 0, banded load, stores 0/2
        # u[kj*UL + 91 + t] = K[t, kj] for all 25 taps in one DMA.
        with nc.allow_non_contiguous_dma(reason="25-element tap scatter"):
            nc.sync.dma_start(
                out=_ap(u_dram[:], OD - 1, [[UL, KD], [1, KD]]),
                in_=_ap(k_in[:], 0, [[1, KD], [KD, KD]]),
            ).then_inc(sem_scat, 16)

        nc.sync.dma_start(
            out=x_sb[:, 0:HALF, :],
            in_=_ap(x_in[:], 0, [[IN, IN], [ISIZE, HALF], [1, IN]]),
        ).then_inc(sem_x0, 16)

        # B_tmp[p, kj, r] = u[kj*UL + p + r]  (= B[p, kj, 91-r]), all 5
        # stripes in a single overlapping-window DMA read.
        nc.sync.wait_ge(sem_scat, 16)
        nc.sync.dma_start(
            out=b_tmp[:],
            in_=_ap(u_dram[:], 0, [[1, IN], [UL, KD], [1, OD]]),
        ).then_inc(sem_bt, 16)

        # ---- scalar ring: x half 1, stores 1/3
        nc.scalar.dma_start(
            out=x_sb[:, HALF:BPC, :],
            in_=_ap(x_in[:], HALF * ISIZE, [[IN, IN], [ISIZE, HALF], [1, IN]]),
        ).then_inc(sem_x1, 16)

        # ---- vector: f32r casts; band reversal fused into one strided cast
        nc.vector.wait_ge(sem_x0, 16)
        nc.vector.tensor_copy(
            x_r[:, 0:HALF, :], x_sb[:, 0:HALF, :]
        ).then_inc(sem_xr0, 1)
        nc.vector.wait_ge(sem_bt, 16)
        # b_sb[p, kj, oi] = b_tmp[p, kj, 91-oi]
        nc.vector.tensor_copy(
            b_sb[:],
            _ap(b_tmp[:], OD - 1, [[KD * OD, IN], [OD, KD], [-1, OD]]),
        ).then_inc(sem_brev, 1)
        nc.vector.wait_ge(sem_x1, 16)
        nc.vector.tensor_copy(
            x_r[:, HALF:BPC, :], x_sb[:, HALF:BPC, :]
        ).then_inc(sem_xr1, 1)

        # ---- tensor: h-outer accumulated f32r matmuls
        nc.tensor.wait_ge(sem_xr0, 1)
        nc.tensor.wait_ge(sem_brev, 1)
        for h in range(2):
            if h == 1:
                nc.tensor.wait_ge(sem_xr1, 1)
            for kj in range(KD):
                mm = nc.tensor.matmul(
                    psums[h][:],
                    b_sb[:, kj, :],
                    _ap(
                        x_r[:],
                        h * HALF * IN + kj,
                        [[BPC * IN, IN], [IN, HALF], [1, OD]],
                    ),
                    start=(kj == 0),
                    stop=(kj == KD - 1),
                )
                if kj == KD - 1:
                    mm.then_inc(sem_mm, 1)

        # ---- vector: quarter copies psum -> out_sb (q covers images 2q..2q+1)
        for q in range(4):
            h, lo = q // 2, (q % 2) * QTR
            nc.vector.wait_ge(sem_mm, h + 1)
            nc.vector.tensor_copy(
                out_sb[:, q * QTR : (q + 1) * QTR, :],
                psums[h][:, lo : lo + QTR, :],
            ).then_inc(sem_copy, 1)

        # ---- stores: quarters alternate between the two HWDGE rings
        def store(engine, q):
            engine.wait_ge(sem_copy, q + 1)
            engine.dma_start(
                out=_ap(
                    y_out[:],
                    q * QTR * OSIZE,
                    [[OD, OD], [OSIZE, QTR], [1, OD]],
                ),
                in_=out_sb[:, q * QTR : (q + 1) * QTR, :],
            ).then_inc(sem_y, 16)

        store(nc.sync, 0)
        store(nc.scalar, 1)
        store(nc.sync, 2)
        store(nc.scalar, 3)
        # hold execution open until every store has landed
        nc.sync.wait_ge(sem_y, 64)

    return nc


_NC = None


def kernel(x: np.ndarray, kernel: np.ndarray) -> np.ndarray:
    global _NC
    if _NC is None:
        _NC = _build_program()

    x = np.ascontiguousarray(x, dtype=np.float32)
    k = np.ascontiguousarray(kernel, dtype=np.float32)
    in_maps = [
        {"x": x[c * BPC : (c + 1) * BPC], "k": k} for c in range(NCORES)
    ]
    res = run_bass_kernel_spmd(_NC, in_maps, list(range(NCORES)))
    return np.concatenate([res.results[c]["y"] for c in range(NCORES)], axis=0)


# revision 4
# speedup vs baseline: 1.1321x; 1.1321x over previous
"""Trainium2 Bass kernel for a 5x5 valid convolution over 96x96 images.

Reference computes x @ W.T where W is the [8464, 9216] conv-as-matmul
matrix (10 GFLOP dense).  We compute the convolution directly on the
tensor engine as 5 PSUM-accumulated banded matmuls (row-conv over the
image-row contraction, column shifts folded into the rhs access pattern):

    out[oi, b, oj] = sum_kj  B_kj.T @ X[:, b, oj+kj]
    B_kj[i, oi]    = K[i-oi, kj]   (banded Toeplitz)

The band matrix depends only on the 25-float kernel K, so it is
expanded on the host (like the reference's conv_mat) and passed as a
per-core input: that removes the on-device scatter -> DRAM -> banded
window-read latency chain entirely.  DRAM params are declared float32r
(bit-identical to fp32) so the matmul operands come straight from DMA
with no DVE casts.

Sharding: data-parallel over batch; each of the 8 cores convolves 8
images.  Raw Bass without a Block, hand-scheduled static DAG.  x is
split across both HWDGE rings; stores alternate rings per quarter.
"""

import sys

sys.path.insert(0, "/opt/trn_rl_repo")

import numpy as np

import bass_rust
import concourse.bass as bass
import concourse.mybir as mybir
from concourse.bass_utils import run_bass_kernel_spmd

# Problem geometry (hardcoded per the task contract).
BATCH = 64
IN = 96           # input image side
KD = 5            # conv kernel side
OD = IN - KD + 1  # output side = 92
ISIZE = IN * IN   # 9216
OSIZE = OD * OD   # 8464
NCORES = 8
BPC = BATCH // NCORES  # images per core = 8
HALF = BPC // 2        # images per PSUM accumulation group = 4
QTR = BPC // 4         # images per store quarter = 2


def _ap(view, offset, dims):
    ap = view.copy()
    ap.offset = offset
    ap.ap = bass_rust.VecI64Pair(dims)
    return ap


def _build_program():
    nc = bass.Bass()
    dt = mybir.dt.float32
    f32r = mybir.dt.float32r

    x_in = nc.declare_dram_parameter("x", [BPC, ISIZE], f32r, isOutput=False)
    b_in = nc.declare_dram_parameter("b", [IN, KD * OD], f32r, isOutput=False)
    y_out = nc.declare_dram_parameter("y", [BPC, OSIZE], dt, isOutput=True)

    from contextlib import ExitStack

    with ExitStack() as ctx:
        b_sb = ctx.enter_context(nc.sbuf_tensor("b_sb", [IN, KD, OD], f32r))
        x_sb = ctx.enter_context(nc.sbuf_tensor("x_sb", [IN, BPC, IN], f32r))
        out_sb = ctx.enter_context(nc.sbuf_tensor("out_sb", [OD, BPC, OD], dt))
        ps0 = ctx.enter_context(nc.psum_tensor("ps0", [OD, HALF, OD], dt))
        ps1 = ctx.enter_context(nc.psum_tensor("ps1", [OD, HALF, OD], dt))
        sem = lambda n: ctx.enter_context(nc.semaphore(n))
        sem_b = sem("sem_b")          # band matrix -> b_sb
        sem_x0 = sem("sem_x0")        # images 0..3 -> x_sb
        sem_x1 = sem("sem_x1")        # images 4..7 -> x_sb
        sem_mm = sem("sem_mm")        # psum group done
        sem_copy = sem("sem_copy")    # psum -> out_sb quarter done
        sem_y = sem("sem_y")          # out_sb -> y

        psums = [ps0, ps1]

        # ---- sync ring: x half 0, stores 0/2
        nc.sync.dma_start(
            out=x_sb[:, 0:HALF, :],
            in_=_ap(x_in[:], 0, [[IN, IN], [ISIZE, HALF], [1, IN]]),
        ).then_inc(sem_x0, 16)

        # ---- scalar ring: band matrix (gates h0), x half 1, stores 1/3
        nc.scalar.dma_start(out=b_sb[:], in_=b_in[:]).then_inc(sem_b, 16)
        nc.scalar.dma_start(
            out=x_sb[:, HALF:BPC, :],
            in_=_ap(x_in[:], HALF * ISIZE, [[IN, IN], [ISIZE, HALF], [1, IN]]),
        ).then_inc(sem_x1, 16)

        # ---- tensor: h-outer accumulated f32r matmuls
        nc.tensor.wait_ge(sem_b, 16)
        nc.tensor.wait_ge(sem_x0, 16)
        for h in range(2):
            if h == 1:
                nc.tensor.wait_ge(sem_x1, 16)
            for kj in range(KD):
                mm = nc.tensor.matmul(
                    psums[h][:],
                    b_sb[:, kj, :],
                    _ap(
                        x_sb[:],
                        h * HALF * IN + kj,
                        [[BPC * IN, IN], [IN, HALF], [1, OD]],
                    ),
                    start=(kj == 0),
                    stop=(kj == KD - 1),
                )
                if kj == KD - 1:
                    mm.then_inc(sem_mm, 1)

        # ---- vector: quarter copies psum -> out_sb (q covers images 2q..2q+1)
        for q in range(4):
            h, lo = q // 2, (q % 2) * QTR
            nc.vector.wait_ge(sem_mm, h + 1)
            nc.vector.tensor_copy(
                out_sb[:, q * QTR : (q + 1) * QTR, :],
                psums[h][:, lo : lo + QTR, :],
            ).then_inc(sem_copy, 1)

        # ---- stores: quarters alternate between the two HWDGE rings
        def store(engine, q):
            engine.wait_ge(sem_copy, q + 1)
            engine.dma_start(
                out=_ap(
                    y_out[:],
                    q * QTR * OSIZE,
                    [[OD, OD], [OSIZE, QTR], [1, OD]],
                ),
                in_=out_sb[:, q * QTR : (q + 1) * QTR, :],
            ).then_inc(sem_y, 16)

        store(nc.sync, 0)
        store(nc.scalar, 1)
        store(nc.sync, 2)
        store(nc.scalar, 3)
        # hold execution open until every store has landed
        nc.sync.wait_ge(sem_y, 64)

    return nc


def _band_matrix(k: np.ndarray) -> np.ndarray:
    """Pre-reversed banded Toeplitz: b[i, kj, oi] = K[i-oi, kj]."""
    b = np.zeros((IN, KD, OD), np.float32)
    oi = np.arange(OD)
    for t in range(KD):
        for kj in range(KD):
            b[oi + t, kj, oi] = k[t, kj]
    return b.reshape(IN, KD * OD)


_NC = None


def kernel(x: np.ndarray, kernel: np.ndarray) -> np.ndarray:
    global _NC
    if _NC is None:
        _NC = _build_program()

    x = np.ascontiguousarray(x, dtype=np.float32)
    k = np.ascontiguousarray(kernel, dtype=np.float32)
    b = _band_matrix(k)
    in_maps = [
        {"x": x[c * BPC : (c + 1) * BPC], "b": b} for c in range(NCORES)
    ]
    res = run_bass_kernel_spmd(_NC, in_maps, list(range(NCORES)))
    return np.concatenate([res.results[c]["y"] for c in range(NCORES)], axis=0)
